# revision 1
# baseline (speedup 1.0000x reference)
"""GAT-style attention head, distributed across 8 TRN2 NeuronCores.

Math (per batch b):
    S   = seq @ Wf                     [N, D]
    f1  = S @ w1 + b1                  [N]
    f2  = S @ w2 + b2                  [N]
    t   = f1[:, None] + f2[None, :]    [N, N]
    e   = exp(leaky_relu(t, 0.2)) = max(exp(t), exp(0.2 t))
        = max(exp(f1_i) * exp(f2'_j), exp(0.2 f1_i) * exp(0.2 f2'_j))
    out = leaky_relu((e @ S) / rowsum(e) + bias, 0.2)

Sharding: rows (i) split across 8 cores; every core needs full S and f2
(one fused AllGather of [S as bf16 | f2 as f32-bitcast]).

Layout trick: everything elementwise is computed in e^T layout
[j_in_chunk(128 partitions), i(free)], which feeds the TensorEngine
directly as stationary weights; rhs is [S_chunk | ones] so one matmul
accumulation produces both e@S and rowsum(e).
"""

import os
import sys
import numpy as np

if "/opt/trn_rl_repo" not in sys.path:
    sys.path.insert(0, "/opt/trn_rl_repo")

B, N, F, D = 2, 8192, 256, 128
CORES = 8
NL = N // CORES          # 1024 rows per core per batch
JC = N // 128            # 64 j-chunks per batch
IT = NL // 128           # 8 i-tiles per core per batch
ALPHA = 0.2

S_ELEMS = B * NL * D         # S payload in AG block, bf16 elems (262144)
F2_BF16 = B * NL * 2         # f2 (f32) viewed as bf16 elems (4096)
BLK = S_ELEMS + F2_BF16      # per-rank AG block, bf16 elems

_cache = {}


def build(skip_collective=False, stop_stage=99, mm_only=False, no_mm=False, jc_lim=None, no_stt=False, no_act=False):
    import concourse.bass as bass
    import concourse.bacc as bacc
    import concourse.mybir as mybir
    import concourse.tile as tile
    from concourse.masks import make_identity

    f32 = mybir.dt.float32
    bf16 = mybir.dt.bfloat16
    AF = mybir.ActivationFunctionType
    ALU = mybir.AluOpType

    nc = bacc.Bacc(None, debug=False, num_devices=CORES)

    seq_ext = nc.declare_dram_parameter("seq", [B, NL, F], f32, isOutput=False)
    wf_ext = nc.declare_dram_parameter("Wf", [F, D], f32, isOutput=False)
    w1_ext = nc.declare_dram_parameter("w1", [D, 1], f32, isOutput=False)
    b1_ext = nc.declare_dram_parameter("b1", [1], f32, isOutput=False)
    w2_ext = nc.declare_dram_parameter("w2", [D, 1], f32, isOutput=False)
    b2_ext = nc.declare_dram_parameter("b2", [1], f32, isOutput=False)
    bias_ext = nc.declare_dram_parameter("bias", [D], f32, isOutput=False)
    out_ext = nc.declare_dram_parameter("out", [B, NL, D], f32, isOutput=True)

    with tile.TileContext(nc) as tc:
        persist_pool = tc.tile_pool(name="persist", bufs=1)
        pers = persist_pool.__enter__()

        def T(shape, dtype, name):
            return pers.tile(shape, dtype, tag=name, name=name)

        with tc.tile_pool(name="dram", bufs=1, space="DRAM") as dram:
            ag_in = dram.tile([BLK], bf16)
            ag_out = dram.tile(
                [CORES * BLK], bf16,
                addr_space=("Local" if skip_collective else "Shared"),
                name="ag_out",
            )

            # ---------- persistent SBUF tensors ----------
            wf_sb = T([128, F], f32, name="wf_sb")         # [f_in_chunk, (fc, d)] -> Wf rows
            w1_sb = T([128, 1], f32, name="w1_sb")
            w2_sb = T([128, 1], f32, name="w2_sb")
            scal = T([128, 4], f32, name="scal")
            b1_sb = scal[0:1, 0:1]
            b2_sb = scal[0:1, 1:2]
            bias_row = T([1, D], f32, name="bias_row")
            ident = T([128, 128], f32, name="ident")
            ones_col = T([1, 128], f32, name="ones_col")

            xt = T([128, B, 2, NL], f32, name="xt")     # X^T: [f, b, fc, n]
            s_stage = T([128, B, IT, D], bf16, name="s_stage")   # S natural (bf16) for AG
            st_sb = T([128, B * NL], f32, name="st_sb")    # S^T: [d, (b, n)]
            f1_sb = T([1, B * NL], f32, name="f1_sb")
            f2_sb = T([1, B * NL], f32, name="f2_sb")
            f2c = T([128, B * JC], f32, name="f2c")      # f2' per-partition cols
            d_cols = T([128, B * JC], f32, name="d_cols")   # exp(0.2 f2')
            b12 = scal[0:1, 2:3]
            b12_bc = scal[:, 3:4]
            f1_bc = T([128, B * NL], f32, name="f1_bc")    # f1 broadcast along partitions
            c_bc = T([128, B * NL], bf16, name="c_bc")    # exp(0.2 f1) broadcast
            bias_bc = T([128, D], f32, name="bias_bc")
            sa0 = T([128, JC * (D + 1)], bf16, name="sa0")   # [S_chunk | ones] batch 0
            sa1 = T([128, JC * (D + 1)], bf16, name="sa1")   # batch 1
            sa = [sa0, sa1]

            # ---------- load small inputs ----------
            for fc in range(2):
                nc.sync.dma_start(
                    out=wf_sb[:, fc * D:(fc + 1) * D],
                    in_=wf_ext[fc * 128:(fc + 1) * 128, :],
                )
            nc.sync.dma_start(out=w1_sb[:, :], in_=w1_ext[:, :])
            nc.sync.dma_start(out=w2_sb[:, :], in_=w2_ext[:, :])
            nc.sync.dma_start(out=b1_sb, in_=b1_ext[:].unsqueeze(0))
            nc.sync.dma_start(out=b2_sb, in_=b2_ext[:].unsqueeze(0))
            nc.sync.dma_start(out=bias_row[:, :], in_=bias_ext[:].unsqueeze(0))
            make_identity(nc, ident[:, :])
            nc.vector.memset(ones_col[:, :], 1.0)

            # ---------- phase 0: load X naturally, PE-transpose into xt ----------
            with (
                tc.tile_pool(name="xn_pool", bufs=3) as xn_pool,
                tc.tile_pool(name="ph_psum", bufs=1, space="PSUM") as php,
            ):
                for b in range(B):
                    for nt in range(IT):
                        xn = xn_pool.tile([128, F], f32, tag="xn")
                        nc.sync.dma_start(
                            out=xn[:, :],
                            in_=seq_ext[b, nt * 128:(nt + 1) * 128, :],
                        )
                        for fc in range(2):
                            pt = php.tile([128, 128], f32, tag="mm128", bufs=2, name="pt")
                            nc.tensor.transpose(
                                pt[:, :], xn[:, fc * 128:(fc + 1) * 128], ident[:, :]
                            )
                            nc.scalar.copy(
                                out=xt[:, b, fc, nt * 128:(nt + 1) * 128],
                                in_=pt[:, :],
                            )

                # ---------- phase 1: S matmuls ----------
                # S natural (per 128-row tile): psum = xt_chunk^T @ Wf_chunk
                for b in range(B):
                    for nt in range(IT):
                        ps = php.tile([128, D], f32, tag="mm128", bufs=2, name="ps")
                        for fc in range(2):
                            nc.tensor.matmul(
                                ps[:, :],
                                lhsT=xt[:, b, fc, nt * 128:(nt + 1) * 128],
                                rhs=wf_sb[:, fc * D:(fc + 1) * D],
                                start=(fc == 0),
                                stop=(fc == 1),
                            )
                        nc.scalar.copy(
                            out=s_stage[:, b, nt, :], in_=ps[:, :]
                        )

                # S^T: psum[d, 512-rows] = Wf_chunk^T(lhsT) @ xt_chunk
                for b in range(B):
                    for h in range(2):
                        pst = php.tile([128, 512], f32, tag="p512", bufs=2, name="pst")
                        for fc in range(2):
                            nc.tensor.matmul(
                                pst[:, :],
                                lhsT=wf_sb[:, fc * D:(fc + 1) * D],
                                rhs=xt[:, b, fc, h * 512:(h + 1) * 512],
                                start=(fc == 0),
                                stop=(fc == 1),
                            )
                        nc.scalar.copy(
                            out=st_sb[:, b * NL + h * 512: b * NL + (h + 1) * 512],
                            in_=pst[:, :],
                        )

                # f1 = w1^T @ S^T, f2 = w2^T @ S^T   (row vectors [1, B*NL])
                for seg in range(B * NL // 512):
                    sl = slice(seg * 512, (seg + 1) * 512)
                    pf1 = php.tile([1, 512], f32, tag="pf", bufs=2, name="pf1")
                    nc.tensor.matmul(pf1[:, :], lhsT=w1_sb[:, :], rhs=st_sb[:, sl])
                    nc.scalar.copy(out=f1_sb[:, sl], in_=pf1[:, :])
                    pf2 = php.tile([1, 512], f32, tag="pf", bufs=2, name="pf2")
                    nc.tensor.matmul(pf2[:, :], lhsT=w2_sb[:, :], rhs=st_sb[:, sl])
                    nc.scalar.copy(out=f2_sb[:, sl], in_=pf2[:, :])

                # f1 broadcast to 128 partitions via PE ones-outer-product
                for seg in range(B * NL // 512):
                    sl = slice(seg * 512, (seg + 1) * 512)
                    pb = php.tile([128, 512], f32, tag="p512", bufs=2, name="pb")
                    nc.tensor.matmul(pb[:, :], lhsT=ones_col[:, :], rhs=f1_sb[:, sl])
                    nc.scalar.copy(out=f1_bc[:, sl], in_=pb[:, :])
                # bias broadcast [128, D]
                pbb = php.tile([128, D], f32, tag="mm128", bufs=2, name="pbb")
                nc.tensor.matmul(pbb[:, :], lhsT=ones_col[:, :], rhs=bias_row[:, :])
                nc.scalar.copy(out=bias_bc[:, :], in_=pbb[:, :])

            # c = exp(0.2 * f1) broadcast (bf16)
            for h in range(2):
                hs = slice(h * NL, (h + 1) * NL)
                nc.scalar.activation(c_bc[:, hs], f1_bc[:, hs], AF.Exp, scale=ALPHA)

            # ---------- AG payload: S (bf16) + f2 (f32 bitcast) ----------
            nc.sync.dma_start(
                out=ag_in[0:S_ELEMS].rearrange(
                    "(b nt p d) -> p b nt d", b=B, nt=IT, p=128, d=D
                ),
                in_=s_stage[:, :, :, :],
            )
            nc.sync.dma_start(
                out=ag_in[S_ELEMS:BLK].bitcast(f32),
                in_=f2_sb[:, :],
            )
            if skip_collective:
                for r in range(CORES):
                    nc.sync.dma_start(
                        out=ag_out[r * BLK:(r + 1) * BLK], in_=ag_in[:]
                    )
            else:
                nc.gpsimd.collective_compute(
                    "AllGather",
                    ALU.bypass,
                    replica_groups=[list(range(CORES))],
                    ins=[ag_in[:].opt()],
                    outs=[ag_out[:].opt()],
                )

            # ---------- unpack gathered S into [S_chunk | ones] tiles ----------
            W = D + 1
            for b in range(B):
                nc.vector.memset(sa[b][:, :], 1.0)
            for b in range(B):
                sav = sa[b].rearrange("p (jc w) -> p jc w", w=W)
                for r in range(CORES):
                    base = r * BLK + b * NL * D
                    nc.sync.dma_start(
                        out=sav[:, r * 8:(r + 1) * 8, 0:D],
                        in_=ag_out[base: base + NL * D].rearrange(
                            "(cl p d) -> p cl d", p=128, d=D
                        ),
                    )

            # gathered f2 -> per-partition columns f2c[p, b*JC + r*8 + cl]
            agf = ag_out[:].bitcast(f32)
            for b in range(B):
                for r in range(CORES):
                    base = (r * BLK + S_ELEMS) // 2 + b * NL
                    nc.sync.dma_start(
                        out=f2c[:, b * JC + r * 8: b * JC + (r + 1) * 8],
                        in_=agf[base: base + NL].rearrange(
                            "(cl p) -> p cl", p=128
                        ),
                    )

            # f2' = f2 + (b1 + b2); d = exp(0.2 f2')
            nc.vector.tensor_tensor(
                out=b12, in0=b1_sb, in1=b2_sb, op=ALU.add
            )
            nc.gpsimd.partition_broadcast(b12_bc, b12)
            nc.vector.tensor_scalar_add(f2c[:, :], f2c[:, :], b12_bc)
            nc.scalar.activation(d_cols[:, :], f2c[:, :], AF.Exp, scale=ALPHA)

            # ---------- main loop ----------
            with (
                tc.tile_pool(name="u_pool", bufs=4) as u_pool,
                tc.tile_pool(name="e_pool", bufs=4) as e_pool,
                tc.tile_pool(name="o_pool", bufs=4) as o_pool,
                tc.tile_pool(name="mm_psum", bufs=1, space="PSUM") as pmm,
            ):
                JCL = JC if jc_lim is None else jc_lim
                for b in range(B):
                    isl = slice(b * NL, (b + 1) * NL)
                    po = [
                        pmm.tile([128, W], f32, tag=f"po{it}", bufs=1, name=f"po{it}")
                        for it in range(IT)
                    ]
                    for jc in range(JCL):
                        col = b * JC + jc
                        if not mm_only:
                            u = u_pool.tile([128, NL], bf16, tag="u")
                            if no_act:
                                nc.vector.memset(u[:, :], 0.25)
                            else:
                                nc.scalar.activation(
                                    u[:, :], f1_bc[:, isl], AF.Exp,
                                    bias=f2c[:, col:col + 1], scale=1.0,
                                )
                            e = e_pool.tile([128, NL], bf16, tag="e")
                            # DVE SBUF reads >512 free-dim hang in this
                            # environment -- split into 512-wide halves.
                            for h in range(2):
                                hs = slice(h * 512, (h + 1) * 512)
                                nc.vector.scalar_tensor_tensor(
                                    out=e[:, hs],
                                    in0=c_bc[:, b * NL + h * 512:
                                             b * NL + (h + 1) * 512],
                                    scalar=d_cols[:, col:col + 1],
                                    in1=u[:, hs],
                                    op0=ALU.mult,
                                    op1=ALU.max,
                                )
                        else:
                            if os.environ.get("TWO_MEMSETS"):
                                u = u_pool.tile([128, NL], bf16, tag="u")
                                nc.vector.memset(u[:, :], 0.25)
                            e = e_pool.tile([128, NL], bf16, tag="e")
                            if os.environ.get("E_FROM_F1"):
                                nc.scalar.copy(out=e[:, :], in_=f1_bc[:, isl])
                            elif os.environ.get("E_FROM_C_SPLIT"):
                                nc.vector.tensor_copy(
                                    e[:, 0:512], c_bc[:, b * NL: b * NL + 512])
                                nc.vector.tensor_copy(
                                    e[:, 512:1024],
                                    c_bc[:, b * NL + 512: b * NL + 1024])
                            elif os.environ.get("E_FROM_C"):
                                nc.vector.tensor_copy(e[:, :], c_bc[:, isl])
                            else:
                                nc.vector.memset(e[:, :], 0.5)
                        if no_mm:
                            continue
                        for it in range(IT):
                            nc.tensor.matmul(
                                po[it][:, :],
                                lhsT=e[:, it * 128:(it + 1) * 128],
                                rhs=sa[b][:, jc * W:(jc + 1) * W],
                                start=(jc == 0),
                                stop=(jc == JCL - 1),
                                skip_group_check=True,
                            )
                    # epilogue
                    for it in range(0 if no_mm else IT):
                        zr = o_pool.tile([128, 1], f32, tag="zr")
                        nc.vector.reciprocal(zr[:, :], po[it][:, D:D + 1])
                        y = o_pool.tile([128, D], f32, tag="y")
                        nc.vector.scalar_tensor_tensor(
                            out=y[:, :],
                            in0=po[it][:, 0:D],
                            scalar=zr[:, 0:1],
                            in1=bias_bc[:, :],
                            op0=ALU.mult,
                            op1=ALU.add,
                        )
                        y2 = o_pool.tile([128, D], f32, tag="y2")
                        nc.vector.tensor_scalar_mul(y2[:, :], y[:, :], ALPHA)
                        o = o_pool.tile([128, D], f32, tag="o")
                        nc.vector.tensor_tensor(
                            out=o[:, :], in0=y[:, :], in1=y2[:, :], op=ALU.max
                        )
                        nc.sync.dma_start(
                            out=out_ext[b, it * 128:(it + 1) * 128, :],
                            in_=o[:, :],
                        )

        persist_pool.__exit__(None, None, None)

    nc.compile()
    return nc


def _get_nc():
    if "nc" not in _cache:
        _cache["nc"] = build()
    return _cache["nc"]


def kernel(seq, Wf, w1, b1, w2, b2, bias):
    from concourse.bass_utils import run_bass_kernel_spmd

    seq = np.ascontiguousarray(np.asarray(seq, dtype=np.float32))
    Wf = np.ascontiguousarray(np.asarray(Wf, dtype=np.float32))
    w1 = np.ascontiguousarray(np.asarray(w1, dtype=np.float32))
    b1 = np.ascontiguousarray(np.asarray(b1, dtype=np.float32))
    w2 = np.ascontiguousarray(np.asarray(w2, dtype=np.float32))
    b2 = np.ascontiguousarray(np.asarray(b2, dtype=np.float32))
    bias = np.ascontiguousarray(np.asarray(bias, dtype=np.float32))

    nc = _get_nc()
    in_maps = []
    for r in range(CORES):
        in_maps.append({
            "seq": np.ascontiguousarray(seq[:, r * NL:(r + 1) * NL, :]),
            "Wf": Wf, "w1": w1, "b1": b1, "w2": w2, "b2": b2, "bias": bias,
        })

    trace = bool(int(os.environ.get("KERNEL_TRACE", "0")))
    if trace:
        import concourse.bass_utils as bu
        bu.upload_artifacts = lambda tmpdir: ""  # no network in container

    res = run_bass_kernel_spmd(
        nc, in_maps, core_ids=list(range(CORES)), trace=trace
    )
    _cache["last_result"] = res
    _cache["exec_time_ns"] = res.exec_time_ns

    out = np.concatenate(
        [res.results[r]["out"] for r in range(CORES)], axis=1
    )
    return np.ascontiguousarray(out.astype(np.float32))



# revision 5
# speedup vs baseline: 1.9064x; 1.9064x over previous
"""GAT-style attention head, distributed across 8 TRN2 NeuronCores.

Math (per batch b):
    S   = seq @ Wf                     [N, D]
    F1  = S @ w1 + b1                  [N]
    F2  = S @ w2 + b2                  [N]
    t   = F1[:, None] + F2[None, :]    [N, N]
    e   = exp(leaky_relu(t, 0.2)) = max(exp(t), exp(0.2 t))
    out = leaky_relu((e @ S) / rowsum(e) + bias, 0.2)

Softmax is row-shift invariant, so scale row i by exp(-0.2 F1_i):
    e'_ij = max(g_i * b_j, d_j)
with g = exp(0.8 F1), b = exp(F2), d = exp(0.2 F2).  This turns the
whole NxN elementwise stage into ONE dual-scalar DVE tensor_scalar op
per [128, NL] tile (bf16 4x mode): (g_bc * b_scalar) max d_scalar.

Sharding: rows (i) split across 8 cores; every core needs full S and F2.
Per-batch fused AllGather of [S bf16 | f2 f32-bitcast], both stored
partition-major ([p, nt, d] / [p, nt]) so pack and unpack DMAs move
contiguous 2KB runs per partition.  AG of batch 0 overlaps batch 1's
S computation; AG of batch 1 overlaps batch 0's main loop.

Main loop per (b, jc): e' tile [128 j, 1024 i] via one tensor_scalar,
then 8 matmuls lhsT=e'_chunk, rhs=[S_chunk | ones] accumulate e'@S and
rowsum(e') into per-i-tile PSUM banks.
"""

import os
import sys
import numpy as np

if "/opt/trn_rl_repo" not in sys.path:
    sys.path.insert(0, "/opt/trn_rl_repo")

B, N, F, D = 2, 8192, 256, 128
CORES = 8
NL = N // CORES          # 1024 rows per core per batch
JC = N // 128            # 64 j-chunks per batch
IT = NL // 128           # 8 i-tiles per core per batch
ALPHA = 0.2
W = D + 1                # S chunk | ones column

S_ELEMS = NL * D         # per-rank per-batch S payload, bf16 elems
F2_BF16 = NL * 2         # per-rank per-batch f2 (f32) as bf16 elems
BLK = S_ELEMS + F2_BF16  # per-rank per-batch AG block, bf16 elems

_cache = {}


def build(skip_collective=False):
    import concourse.bass as bass
    import concourse.bacc as bacc
    import concourse.mybir as mybir
    import concourse.tile as tile
    from concourse.masks import make_identity

    f32 = mybir.dt.float32
    bf16 = mybir.dt.bfloat16
    AF = mybir.ActivationFunctionType
    ALU = mybir.AluOpType

    nc = bacc.Bacc(None, debug=False, num_devices=CORES)

    seq_ext = nc.declare_dram_parameter("seq", [B, NL, F], f32, isOutput=False)
    wf_ext = nc.declare_dram_parameter("Wf", [F, D], f32, isOutput=False)
    w1_ext = nc.declare_dram_parameter("w1", [D, 1], f32, isOutput=False)
    b1_ext = nc.declare_dram_parameter("b1", [1], f32, isOutput=False)
    w2_ext = nc.declare_dram_parameter("w2", [D, 1], f32, isOutput=False)
    b2_ext = nc.declare_dram_parameter("b2", [1], f32, isOutput=False)
    bias_ext = nc.declare_dram_parameter("bias", [D], f32, isOutput=False)
    out_ext = nc.declare_dram_parameter("out", [B, NL, D], f32, isOutput=True)

    with tile.TileContext(nc) as tc:
        persist_pool = tc.tile_pool(name="persist", bufs=1)
        pers = persist_pool.__enter__()

        def T(shape, dtype, name):
            return pers.tile(shape, dtype, tag=name, name=name)

        with tc.tile_pool(name="dram", bufs=1, space="DRAM") as dram:
            ag_in = [dram.tile([BLK], bf16, name=f"ag_in{b}") for b in range(B)]
            ag_out = [
                dram.tile(
                    [CORES * BLK], bf16,
                    addr_space=("Local" if skip_collective else "Shared"),
                    name=f"ag_out{b}",
                )
                for b in range(B)
            ]

            # ---------- persistent SBUF tensors ----------
            wf_sb = T([128, F], f32, name="wf_sb")      # [f_chunk, (fc, d)]
            w1_sb = T([128, 1], f32, name="w1_sb")
            w2_sb = T([128, 1], f32, name="w2_sb")
            scal = T([128, 8], f32, name="scal")
            b1_sb = scal[0:1, 0:1]
            b2_sb = scal[0:1, 1:2]
            sb1 = scal[0:1, 2:3]         # 0.8 * b1
            sb2 = scal[0:1, 3:4]         # 0.2 * b2
            sb1_bc = scal[:, 4:5]        # broadcasts over partitions
            b2_bc = scal[:, 5:6]
            sb2_bc = scal[:, 6:7]
            bias_row = T([1, D], f32, name="bias_row")
            ident = T([128, 128], f32, name="ident")
            ones_col = T([1, 128], f32, name="ones_col")

            xt = T([128, 2, NL], f32, name="xt")        # X^T of current batch
            s_stage = [T([128, IT, D], bf16, name=f"s_stage{b}") for b in range(B)]
            st_sb = T([128, NL], f32, name="st_sb")     # S^T of current batch
            f1_sb = T([1, B * NL], f32, name="f1_sb")
            f2t = [T([128, IT], f32, name=f"f2t{b}") for b in range(B)]
            g_bc = T([128, B * NL], bf16, name="g_bc")  # exp(0.8 F1) bcast
            f2c = [T([128, JC], f32, name=f"f2c{b}") for b in range(B)]
            bcol = [T([128, JC], f32, name=f"bcol{b}") for b in range(B)]
            dcol = [T([128, JC], f32, name=f"dcol{b}") for b in range(B)]
            bias_bc = T([128, D], f32, name="bias_bc")
            sa = [T([128, JC * W], bf16, name=f"sa{b}") for b in range(B)]

            # ---------- load small inputs ----------
            for fc in range(2):
                nc.sync.dma_start(
                    out=wf_sb[:, fc * D:(fc + 1) * D],
                    in_=wf_ext[fc * 128:(fc + 1) * 128, :],
                )
            nc.sync.dma_start(out=w1_sb[:, :], in_=w1_ext[:, :])
            nc.sync.dma_start(out=w2_sb[:, :], in_=w2_ext[:, :])
            nc.sync.dma_start(out=b1_sb, in_=b1_ext[:].unsqueeze(0))
            nc.sync.dma_start(out=b2_sb, in_=b2_ext[:].unsqueeze(0))
            nc.sync.dma_start(out=bias_row[:, :], in_=bias_ext[:].unsqueeze(0))
            make_identity(nc, ident[:, :])
            nc.vector.memset(ones_col[:, :], 1.0)
            nc.vector.tensor_scalar_mul(sb1, b1_sb, 0.8)
            nc.vector.tensor_scalar_mul(sb2, b2_sb, ALPHA)
            nc.gpsimd.partition_broadcast(sb1_bc, sb1)
            nc.gpsimd.partition_broadcast(b2_bc, b2_sb)
            nc.gpsimd.partition_broadcast(sb2_bc, sb2)

            with (
                tc.tile_pool(name="xn_pool", bufs=3) as xn_pool,
                tc.tile_pool(name="ph_psum", bufs=1, space="PSUM") as php,
            ):
                # bias broadcast [128, D]
                pbb = php.tile([128, D], f32, tag="mm128", bufs=2, name="pbb")
                nc.tensor.matmul(pbb[:, :], lhsT=ones_col[:, :], rhs=bias_row[:, :])
                nc.scalar.copy(out=bias_bc[:, :], in_=pbb[:, :])

                for b in range(B):
                    # ---- load X tiles, PE-transpose into xt ----
                    for nt in range(IT):
                        xn = xn_pool.tile([128, F], f32, tag="xn")
                        nc.sync.dma_start(
                            out=xn[:, :],
                            in_=seq_ext[b, nt * 128:(nt + 1) * 128, :],
                        )
                        for fc in range(2):
                            pt = php.tile([128, 128], f32, tag="mm128", bufs=2, name="pt")
                            nc.tensor.transpose(
                                pt[:, :], xn[:, fc * 128:(fc + 1) * 128], ident[:, :]
                            )
                            nc.scalar.copy(
                                out=xt[:, fc, nt * 128:(nt + 1) * 128],
                                in_=pt[:, :],
                            )

                    # ---- S natural (bf16, partition-major stage) ----
                    for nt in range(IT):
                        ps = php.tile([128, D], f32, tag="mm128", bufs=2, name="ps")
                        for fc in range(2):
                            nc.tensor.matmul(
                                ps[:, :],
                                lhsT=xt[:, fc, nt * 128:(nt + 1) * 128],
                                rhs=wf_sb[:, fc * D:(fc + 1) * D],
                                start=(fc == 0),
                                stop=(fc == 1),
                            )
                        nc.scalar.copy(out=s_stage[b][:, nt, :], in_=ps[:, :])

                    # ---- S^T ----
                    for h in range(2):
                        pst = php.tile([128, 512], f32, tag="p512", bufs=2, name="pst")
                        for fc in range(2):
                            nc.tensor.matmul(
                                pst[:, :],
                                lhsT=wf_sb[:, fc * D:(fc + 1) * D],
                                rhs=xt[:, fc, h * 512:(h + 1) * 512],
                                start=(fc == 0),
                                stop=(fc == 1),
                            )
                        nc.scalar.copy(
                            out=st_sb[:, h * 512:(h + 1) * 512], in_=pst[:, :]
                        )

                    # ---- f1 row; g = exp(0.8 f1 + 0.8 b1) straight from PSUM ----
                    for seg in range(2):
                        sl = slice(seg * 512, (seg + 1) * 512)
                        pf1 = php.tile([1, 512], f32, tag="pf", bufs=2, name="pf1")
                        nc.tensor.matmul(pf1[:, :], lhsT=w1_sb[:, :], rhs=st_sb[:, sl])
                        nc.scalar.copy(out=f1_sb[:, b * NL + seg * 512:
                                                 b * NL + (seg + 1) * 512],
                                       in_=pf1[:, :])
                    for seg in range(2):
                        pb = php.tile([128, 512], f32, tag="p512", bufs=2, name="pb")
                        nc.tensor.matmul(
                            pb[:, :], lhsT=ones_col[:, :],
                            rhs=f1_sb[:, b * NL + seg * 512: b * NL + (seg + 1) * 512],
                        )
                        nc.scalar.activation(
                            g_bc[:, b * NL + seg * 512: b * NL + (seg + 1) * 512],
                            pb[:, :], AF.Exp, bias=sb1_bc, scale=0.8,
                        )

                    # ---- f2 partition-major: f2t[p, nt] = S_tile @ w2 ----
                    for nt in range(IT):
                        pf2 = php.tile([128, 1], f32, tag="pf2", bufs=2, name="pf2")
                        nc.tensor.matmul(
                            pf2[:, :],
                            lhsT=st_sb[:, nt * 128:(nt + 1) * 128],
                            rhs=w2_sb[:, :],
                        )
                        nc.scalar.copy(out=f2t[b][:, nt:nt + 1], in_=pf2[:, :])

                    # ---- pack + AllGather (straight partition-major copies) ----
                    nc.sync.dma_start(
                        out=ag_in[b][0:S_ELEMS].rearrange(
                            "(p nt d) -> p nt d", p=128, nt=IT, d=D
                        ),
                        in_=s_stage[b][:, :, :],
                    )
                    nc.sync.dma_start(
                        out=ag_in[b][S_ELEMS:BLK].bitcast(f32).rearrange(
                            "(p c) -> p c", p=128, c=IT
                        ),
                        in_=f2t[b][:, :],
                    )
                    if skip_collective:
                        for r in range(CORES):
                            nc.sync.dma_start(
                                out=ag_out[b][r * BLK:(r + 1) * BLK],
                                in_=ag_in[b][:],
                            )
                    else:
                        nc.gpsimd.collective_compute(
                            "AllGather",
                            ALU.bypass,
                            replica_groups=[list(range(CORES))],
                            ins=[ag_in[b][:].opt()],
                            outs=[ag_out[b][:].opt()],
                        )

            # ---------- main loop per batch ----------
            with (
                tc.tile_pool(name="e_pool", bufs=4) as e_pool,
                tc.tile_pool(name="o_pool", bufs=4) as o_pool,
                tc.tile_pool(name="mm_psum", bufs=1, space="PSUM") as pmm,
            ):
                # unpack gathered S into [S_chunk | ones] + f2 columns;
                # both batches issued up front so batch 1's unpack overlaps
                # batch 0's main loop (engine queues are FIFO).
                for b in range(B):
                    sav = sa[b].rearrange("p (jc w) -> p jc w", w=W)
                    nc.vector.memset(sav[:, :, D:D + 1], 1.0)
                    for r in range(CORES):
                        base = r * BLK
                        nc.sync.dma_start(
                            out=sav[:, r * IT:(r + 1) * IT, 0:D],
                            in_=ag_out[b][base: base + S_ELEMS].rearrange(
                                "(p nt d) -> p nt d", p=128, nt=IT, d=D
                            ),
                        )
                        nc.sync.dma_start(
                            out=f2c[b][:, r * IT:(r + 1) * IT],
                            in_=ag_out[b][base + S_ELEMS: base + BLK]
                            .bitcast(f32)
                            .rearrange("(p c) -> p c", p=128, c=IT),
                        )
                    nc.scalar.activation(bcol[b][:, :], f2c[b][:, :], AF.Exp,
                                         bias=b2_bc, scale=1.0)
                    nc.scalar.activation(dcol[b][:, :], f2c[b][:, :], AF.Exp,
                                         bias=sb2_bc, scale=ALPHA)

                for b in range(B):
                    po = [
                        pmm.tile([128, W], f32, tag=f"po{it}", bufs=1, name=f"po{it}")
                        for it in range(IT)
                    ]
                    for jc in range(JC):
                        e = e_pool.tile([128, NL], bf16, tag="e")
                        # DVE SBUF reads >512 free-dim hang in this
                        # environment -- split into 512-wide halves.
                        for h in range(2):
                            hs = slice(h * 512, (h + 1) * 512)
                            nc.vector.tensor_scalar(
                                out=e[:, hs],
                                in0=g_bc[:, b * NL + h * 512:
                                         b * NL + (h + 1) * 512],
                                scalar1=bcol[b][:, jc:jc + 1],
                                scalar2=dcol[b][:, jc:jc + 1],
                                op0=ALU.mult,
                                op1=ALU.max,
                            )
                        for it in range(IT):
                            nc.tensor.matmul(
                                po[it][:, :],
                                lhsT=e[:, it * 128:(it + 1) * 128],
                                rhs=sa[b][:, jc * W:(jc + 1) * W],
                                start=(jc == 0),
                                stop=(jc == JC - 1),
                                skip_group_check=True,
                            )
                    # epilogue
                    for it in range(IT):
                        zr = o_pool.tile([128, 1], f32, tag="zr")
                        nc.vector.reciprocal(zr[:, :], po[it][:, D:D + 1])
                        y = o_pool.tile([128, D], f32, tag="y")
                        nc.vector.scalar_tensor_tensor(
                            out=y[:, :],
                            in0=po[it][:, 0:D],
                            scalar=zr[:, 0:1],
                            in1=bias_bc[:, :],
                            op0=ALU.mult,
                            op1=ALU.add,
                        )
                        o = o_pool.tile([128, D], f32, tag="o")
                        nc.vector.scalar_tensor_tensor(
                            out=o[:, :],
                            in0=y[:, :],
                            scalar=ALPHA,
                            in1=y[:, :],
                            op0=ALU.mult,
                            op1=ALU.max,
                        )
                        nc.sync.dma_start(
                            out=out_ext[b, it * 128:(it + 1) * 128, :],
                            in_=o[:, :],
                        )

        persist_pool.__exit__(None, None, None)

    nc.compile()
    return nc


def _get_nc():
    if "nc" not in _cache:
        _cache["nc"] = build()
    return _cache["nc"]


def kernel(seq, Wf, w1, b1, w2, b2, bias):
    from concourse.bass_utils import run_bass_kernel_spmd

    seq = np.ascontiguousarray(np.asarray(seq, dtype=np.float32))
    Wf = np.ascontiguousarray(np.asarray(Wf, dtype=np.float32))
    w1 = np.ascontiguousarray(np.asarray(w1, dtype=np.float32))
    b1 = np.ascontiguousarray(np.asarray(b1, dtype=np.float32))
    w2 = np.ascontiguousarray(np.asarray(w2, dtype=np.float32))
    b2 = np.ascontiguousarray(np.asarray(b2, dtype=np.float32))
    bias = np.ascontiguousarray(np.asarray(bias, dtype=np.float32))

    nc = _get_nc()
    in_maps = []
    for r in range(CORES):
        in_maps.append({
            "seq": np.ascontiguousarray(seq[:, r * NL:(r + 1) * NL, :]),
            "Wf": Wf, "w1": w1, "b1": b1, "w2": w2, "b2": b2, "bias": bias,
        })

    trace = bool(int(os.environ.get("KERNEL_TRACE", "0")))
    if trace:
        import concourse.bass_utils as bu
        bu.upload_artifacts = lambda tmpdir: ""  # no network in container

    res = run_bass_kernel_spmd(
        nc, in_maps, core_ids=list(range(CORES)), trace=trace
    )
    _cache["last_result"] = res
    _cache["exec_time_ns"] = res.exec_time_ns

    out = np.concatenate(
        [res.results[r]["out"] for r in range(CORES)], axis=1
    )
    return np.ascontiguousarray(out.astype(np.float32))
